# revision 5
# baseline (speedup 1.0000x reference)
"""ChromDropout kernel for one TRN2 chip (8 NeuronCores, data-parallel).

Math (training-mode ChromDropout):
    out[b, g] = x[b, g] * (1 - drop[b, chrom_ids[g]]) * (NUM_CHROMS / N_DROP)
where drop[b, :] marks 4 distinct chromosomes sampled per row with
jax.random.permutation(split(key(42), B)[b], 23)[:4].

Strategy:
  - Host (tiny): derive the per-row drop table exactly as the reference does
    (threefry is platform-deterministic), fold the 5.75 rescale into a
    [B, 23] scale table with values {0, 5.75}, and build a [23, G] one-hot
    of chrom_ids.
  - Device (all the heavy traffic): shard x row-wise across 8 cores. Per
    128-row tile, mask[128, N] = scale_t[23, 128].T @ onehot[23, N] on the
    TensorEngine (exact: one-hot selection), then out = x * mask on the
    VectorEngine. Streaming tiles, memory-bound by design.
"""

import numpy as np

B = 16384
G = 19064
C = 23
ND = 4
SCALE = float(C) / float(ND)  # 5.75, exactly representable
NCORES = 8
BS = B // NCORES  # 2048 rows per core
P = 128
CW = 2383  # column tile width; G = 8 * CW
MM = 512  # matmul moving-free-dim chunk (one PSUM bank of f32)

_CACHED = {}


def _build_nc():
    import concourse.bacc as bacc
    import concourse.mybir as mybir
    from concourse.tile import TileContext

    f32 = mybir.dt.float32
    # Bacc (not raw Bass): its compile() runs move_matmul_waits_to_ldweights +
    # generate_event_semaphores, which split multi-semaphore waits down to the
    # 1-wait-per-instruction TRN2 ISA limit.
    nc = bacc.Bacc("TRN2", target_bir_lowering=False, debug=False)
    x = nc.declare_dram_parameter("x", [BS, G], f32, isOutput=False)
    # columns [0, BS) = per-row scale (transposed), [BS, BS+G) = gene one-hot.
    # One parameter -> one DMA -> a single wait semaphore for every matmul
    # (PE Matmult instructions only support one sync-wait on TRN2).
    tables = nc.declare_dram_parameter("tables", [C, BS + G], f32, isOutput=False)
    out = nc.declare_dram_parameter("out", [BS, G], f32, isOutput=True)

    with TileContext(nc) as tc:
        with (
            tc.tile_pool(name="const", bufs=1) as const_pool,
            tc.tile_pool(name="xp", bufs=6) as xp,
            tc.tile_pool(name="pp", bufs=8, space="PSUM") as pp,
        ):
            tbl = const_pool.tile([C, BS + G], f32, tag="tbl")
            nc.sync.dma_start(tbl[:], tables[:])

            for rt in range(BS // P):  # 16 row tiles
                r0 = rt * P
                for ct in range(G // CW):  # 8 col tiles
                    c0 = ct * CW
                    xt = xp.tile([P, CW], f32, tag="xt")
                    nc.sync.dma_start(xt[:], x[r0 : r0 + P, c0 : c0 + CW])
                    for off in range(0, CW, MM):
                        w = min(MM, CW - off)
                        m = pp.tile([P, MM], f32, tag="m")
                        nc.tensor.matmul(
                            m[:, :w],
                            tbl[:, r0 : r0 + P],
                            tbl[:, BS + c0 + off : BS + c0 + off + w],
                            start=True,
                            stop=True,
                        )
                        nc.vector.tensor_tensor(
                            xt[:, off : off + w],
                            xt[:, off : off + w],
                            m[:, :w],
                            mybir.AluOpType.mult,
                        )
                    nc.sync.dma_start(out[r0 : r0 + P, c0 : c0 + CW], xt[:])
    nc.finalize()  # Bacc.finalize -> compile() (wait splitting etc) + freeze
    return nc


def _host_tables(chrom_ids: np.ndarray) -> tuple[np.ndarray, np.ndarray]:
    """scale_t [23, B] with values {0, 5.75}; onehot [23, G]."""
    import jax

    with jax.default_device(jax.devices("cpu")[0]):
        keys = jax.random.split(jax.random.key(42), B)
        sel = np.asarray(
            jax.vmap(lambda k: jax.random.permutation(k, C)[:ND])(keys)
        )  # [B, 4] int32
    drop = np.zeros((B, C), np.float32)
    drop[np.arange(B)[:, None], sel] = 1.0
    scale = (1.0 - drop) * np.float32(SCALE)  # [B, 23]
    onehot = (
        np.asarray(chrom_ids)[None, :] == np.arange(C, dtype=np.int32)[:, None]
    ).astype(np.float32)  # [23, G]
    return np.ascontiguousarray(scale.T), onehot


def kernel(x: np.ndarray, chrom_ids: np.ndarray, **run_kwargs) -> np.ndarray:
    from concourse.bass_utils import run_bass_kernel_spmd

    x = np.asarray(x)
    scale_t, onehot = _host_tables(chrom_ids)

    if "nc" not in _CACHED:
        _CACHED["nc"] = _build_nc()
    nc = _CACHED["nc"]

    in_maps = [
        {
            "x": np.ascontiguousarray(x[i * BS : (i + 1) * BS]),
            "tables": np.ascontiguousarray(
                np.concatenate(
                    [scale_t[:, i * BS : (i + 1) * BS], onehot], axis=1
                )
            ),
        }
        for i in range(NCORES)
    ]
    res = run_bass_kernel_spmd(nc, in_maps, core_ids=list(range(NCORES)), **run_kwargs)
    out = np.concatenate([np.asarray(r["out"]) for r in res.results], axis=0)
    if "exec_time_ns" in dir(res) and res.exec_time_ns is not None:
        kernel.last_exec_time_ns = res.exec_time_ns
    kernel.last_results = res
    return out


# revision 7
# speedup vs baseline: 1.1955x; 1.1955x over previous
"""ChromDropout kernel for one TRN2 chip (8 NeuronCores, data-parallel).

Math (training-mode ChromDropout):
    out[b, g] = x[b, g] * (1 - drop[b, chrom_ids[g]]) * (NUM_CHROMS / N_DROP)
where drop[b, :] marks 4 distinct chromosomes sampled per row with
jax.random.permutation(split(key(42), B)[b], 23)[:4].

Strategy:
  - Host (tiny): derive the per-row drop table exactly as the reference does
    (threefry is platform-deterministic), fold the 5.75 rescale into a
    [B, 23] scale table with values {0, 5.75}, and build a [23, G] one-hot
    of chrom_ids.
  - Device (all the heavy traffic): shard x row-wise across 8 cores. Per
    128-row tile, mask[128, N] = scale_t[23, 128].T @ onehot[23, N] on the
    TensorEngine (exact: one-hot selection), then out = x * mask on the
    VectorEngine. Streaming tiles, memory-bound by design.
"""

import numpy as np

B = 16384
G = 19064
C = 23
ND = 4
SCALE = float(C) / float(ND)  # 5.75, exactly representable
NCORES = 8
BS = B // NCORES  # 2048 rows per core
P = 128
CW = 2383  # column tile width; G = 8 * CW
MM = 512  # matmul moving-free-dim chunk (one PSUM bank of f32)

_CACHED = {}


def _build_nc():
    import concourse.bacc as bacc
    import concourse.mybir as mybir
    from concourse.tile import TileContext

    f32 = mybir.dt.float32
    # Bacc (not raw Bass): its compile() runs move_matmul_waits_to_ldweights +
    # generate_event_semaphores, which split multi-semaphore waits down to the
    # 1-wait-per-instruction TRN2 ISA limit.
    nc = bacc.Bacc("TRN2", target_bir_lowering=False, debug=False)
    x = nc.declare_dram_parameter("x", [BS, G], f32, isOutput=False)
    # columns [0, BS) = per-row scale (transposed), [BS, BS+G) = gene one-hot.
    # One parameter -> one DMA -> a single wait semaphore for every matmul
    # (PE Matmult instructions only support one sync-wait on TRN2).
    tables = nc.declare_dram_parameter("tables", [C, BS + G], f32, isOutput=False)
    out = nc.declare_dram_parameter("out", [BS, G], f32, isOutput=True)

    with TileContext(nc) as tc:
        with (
            tc.tile_pool(name="const", bufs=1) as const_pool,
            tc.tile_pool(name="xp", bufs=6) as xp,
            tc.tile_pool(name="pp", bufs=8, space="PSUM") as pp,
        ):
            tbl = const_pool.tile([C, BS + G], f32, tag="tbl")
            nc.sync.dma_start(tbl[:], tables[:])
            # bf16 copy for the TensorEngine: fp32 matmul streams at 1/4
            # rate and made PE the bottleneck (91% busy). 0/1/5.75 are all
            # exact in bf16, so the mask (and output) stay bit-identical.
            bf16 = mybir.dt.bfloat16
            tblb = const_pool.tile([C, BS + G], bf16, tag="tblb")
            nc.vector.tensor_copy(tblb[:], tbl[:])

            for rt in range(BS // P):  # 16 row tiles
                r0 = rt * P
                for ct in range(G // CW):  # 8 col tiles
                    c0 = ct * CW
                    xt = xp.tile([P, CW], f32, tag="xt")
                    nc.sync.dma_start(xt[:], x[r0 : r0 + P, c0 : c0 + CW])
                    for off in range(0, CW, MM):
                        w = min(MM, CW - off)
                        m = pp.tile([P, MM], f32, tag="m")
                        nc.tensor.matmul(
                            m[:, :w],
                            tblb[:, r0 : r0 + P],
                            tblb[:, BS + c0 + off : BS + c0 + off + w],
                            start=True,
                            stop=True,
                        )
                        nc.vector.tensor_tensor(
                            xt[:, off : off + w],
                            xt[:, off : off + w],
                            m[:, :w],
                            mybir.AluOpType.mult,
                        )
                    nc.sync.dma_start(out[r0 : r0 + P, c0 : c0 + CW], xt[:])
    nc.finalize()  # Bacc.finalize -> compile() (wait splitting etc) + freeze
    return nc


def _host_tables(chrom_ids: np.ndarray) -> tuple[np.ndarray, np.ndarray]:
    """scale_t [23, B] with values {0, 5.75}; onehot [23, G]."""
    import jax

    with jax.default_device(jax.devices("cpu")[0]):
        keys = jax.random.split(jax.random.key(42), B)
        sel = np.asarray(
            jax.vmap(lambda k: jax.random.permutation(k, C)[:ND])(keys)
        )  # [B, 4] int32
    drop = np.zeros((B, C), np.float32)
    drop[np.arange(B)[:, None], sel] = 1.0
    scale = (1.0 - drop) * np.float32(SCALE)  # [B, 23]
    onehot = (
        np.asarray(chrom_ids)[None, :] == np.arange(C, dtype=np.int32)[:, None]
    ).astype(np.float32)  # [23, G]
    return np.ascontiguousarray(scale.T), onehot


def kernel(x: np.ndarray, chrom_ids: np.ndarray, **run_kwargs) -> np.ndarray:
    from concourse.bass_utils import run_bass_kernel_spmd

    x = np.asarray(x)
    scale_t, onehot = _host_tables(chrom_ids)

    if "nc" not in _CACHED:
        _CACHED["nc"] = _build_nc()
    nc = _CACHED["nc"]

    in_maps = [
        {
            "x": np.ascontiguousarray(x[i * BS : (i + 1) * BS]),
            "tables": np.ascontiguousarray(
                np.concatenate(
                    [scale_t[:, i * BS : (i + 1) * BS], onehot], axis=1
                )
            ),
        }
        for i in range(NCORES)
    ]
    res = run_bass_kernel_spmd(nc, in_maps, core_ids=list(range(NCORES)), **run_kwargs)
    out = np.concatenate([np.asarray(r["out"]) for r in res.results], axis=0)
    if "exec_time_ns" in dir(res) and res.exec_time_ns is not None:
        kernel.last_exec_time_ns = res.exec_time_ns
    kernel.last_results = res
    return out
